# revision 9
# baseline (speedup 1.0000x reference)
"""Multi-head self-attention Trainium2 kernel (8 NeuronCores), v4.

Sharding: 8 cores = 4 batches x 2 head-groups (8 heads each).
Core c handles batch b=c//2, heads [g*8, (g+1)*8) where g=c%2.
Host sums the two partials per batch and adds bo.

v4 design (vs the 512us v3 baseline):
- v3 was ACT-bound in phase B (256 exp back-to-back = 332us) behind a
  fully serial 127us projection phase. v4 software-pipelines the
  projections under the attention phase: x lives in SBUF, projections
  are emitted as small per-pair units drained into the pair-major
  attention stage loop, so the exp stream starts ~15us in.
- A fraction of exp stages runs on the Vector engine via a Schraudolph
  bit trick: bf16_bits = rint_i16(s * 128*log2e*scale + 128*(127-c)),
  written by tensor_scalar into an int16-bitcast view of the bf16 ptp
  tile (HW converts round-half-even; ~1.9% rms exp error on those
  stages). Those stages are placed mid-window (jt 5/8/11/14) so the
  norm-burst at window boundaries never queues ahead of them on DVE.
- Output projection packs each head pair vertically: ctx_even in
  partitions 0-63, ctx_odd in 64-127 (odd half placed by a SBUF->SBUF
  DMA after normalize), so C matmuls contract K=128 (4 MMs/out tile).
- PSUM: st 2x[128,1024] (4 banks) + ct 2x[128,512] (2) + one shared
  2-bank slot whose 512-col halves are ping-ponged by the projection
  accumulators (early) and the output-projection tiles (late) = 8 banks.
  PV lags 6 stages and each window's PV/norm fully drains by its last
  stage, so ct bank recycling never stalls the in-order PE queue.
- DVE-exp stages sit mid-window, few in the PE-paced early pairs and
  more in the ACT-paced late pairs; odd output-projection evacuations
  ride the scalar engine, which is idle by then.
"""

import numpy as np

import concourse.bass as bass
import concourse.tile as tile
from concourse import bacc, mybir
from contextlib import ExitStack

P = 128
D = 1024
HD = 512   # head dims per core (8 heads x 64)
NPAIR = 4
NH = 8
VW = 65    # head block width in VS (64 + ones col)
F32 = mybir.dt.float32
BF16 = mybir.dt.bfloat16
I16 = mybir.dt.int16
EXPF = mybir.ActivationFunctionType.Exp
MULT = mybir.AluOpType.mult
ADD = mybir.AluOpType.add

# Schraudolph constants: bf16_bits = rint(s * SCHRA_A + SCHRA_B);
# exp(0.125*s) ~= bitcast_bf16(bits). c calibrated to zero the mean rel err.
SCHRA_C = 0.0577
SCHRA_A = 128.0 * 1.4426950408889634 * 0.125
SCHRA_B = 128.0 * (127.0 - SCHRA_C)

# jt positions whose exp runs on DVE (mid-window, away from norm bursts).
# Early pairs (0,1) are PE-paced and DVE carries the projection evacs, so
# few stages go to DVE; late pairs (2,3) are ACT-paced with an idle DVE,
# so more exp moves there to compress the window.
DVE_JTS = {
    0: {0: frozenset(), 1: frozenset(), 2: frozenset(), 3: frozenset()},
    3: {0: frozenset((8, 12)), 1: frozenset((8, 12)),
        2: frozenset((3, 5, 7, 9, 11, 13)),
        3: frozenset((3, 5, 7, 9, 11, 13))},
    4: {p: frozenset((4, 6, 8, 10, 12, 14)) for p in range(4)},
}


def build_nc(S=2048, dve_mode=3):
    NKT = D // P          # 8 k-tiles over model dim
    NJT = S // P          # 16 key tiles
    QB = 512
    NQB = S // QB         # 4
    MSEG = 512
    NMSEG = S // MSEG     # 4
    PVLAG = 6
    dve_jts = DVE_JTS[dve_mode]

    nc = bacc.Bacc("TRN2", target_bir_lowering=False, debug=False)
    xT = nc.dram_tensor("xT", [D, S], BF16, kind="ExternalInput").ap()
    wq = nc.dram_tensor("wq", [D, HD], BF16, kind="ExternalInput").ap()
    wk = nc.dram_tensor("wk", [D, HD], BF16, kind="ExternalInput").ap()
    wv = nc.dram_tensor("wv", [D, HD], BF16, kind="ExternalInput").ap()
    wo = nc.dram_tensor("wo", [HD, D], BF16, kind="ExternalInput").ap()
    out = nc.dram_tensor("out", [S, D], F32, kind="ExternalOutput").ap()

    with tile.TileContext(nc) as tc:
        with ExitStack() as persist:
            data_pool = persist.enter_context(tc.tile_pool(name="data", bufs=1))

            # --- persistent SBUF tiles ---
            xsb = [data_pool.tile([P, S], BF16, tag=f"x{k}", name=f"x{k}")
                   for k in range(NKT)]
            QT = [data_pool.tile([P, S], BF16, tag=f"qt{p}", name=f"qt{p}")
                  for p in range(NPAIR)]
            KT = [data_pool.tile([P, S], BF16, tag=f"kt{p}", name=f"kt{p}")
                  for p in range(NPAIR)]
            # per-pair V: [128 tokens, 2 x (64 + ones)] per key tile
            VS = [[data_pool.tile([P, 2 * VW], BF16, tag=f"vs{p}_{j}",
                                  name=f"vs{p}_{j}") for j in range(NJT)]
                  for p in range(NPAIR)]
            wq_t = data_pool.tile([P, NKT, HD], BF16, tag="wq", name="wq_t")
            wk_t = data_pool.tile([P, NKT, HD], BF16, tag="wk", name="wk_t")
            wv_t = data_pool.tile([P, NKT, HD], BF16, tag="wv", name="wv_t")
            # stacked pair output weights: rows 0-63 = wo[head 2p], 64-127 = wo[2p+1]
            wo_p = [data_pool.tile([P, D], BF16, tag=f"wo{p}", name=f"wo{p}")
                    for p in range(NPAIR)]
            # per (pair, qb) normalized context, pair-stacked on partitions
            cth = [[data_pool.tile([P, QB], BF16, tag=f"ct{p}_{q}",
                                   name=f"ct{p}_{q}") for q in range(NQB)]
                   for p in range(NPAIR)]

            # --- preload DMAs, ordered for earliest A1(p0) start ---
            def dma_w(dst, src, p):
                cs = slice(p * P, (p + 1) * P)
                nc.sync.dma_start(dst[:, :, cs],
                                  src[:, cs].rearrange("(kt p) n -> p kt n", p=P))

            def dma_x(k, quarter):
                hs = slice(quarter * (S // 4), (quarter + 1) * (S // 4))
                nc.sync.dma_start(xsb[k][:, hs], xT[k * P:(k + 1) * P, hs])

            dma_w(wq_t, wq, 0)
            dma_w(wk_t, wk, 0)
            for q in range(2):
                for k in range(NKT):
                    dma_x(k, q)
            dma_w(wv_t, wv, 0)
            dma_w(wv_t, wv, 1)
            dma_w(wq_t, wq, 1)
            dma_w(wk_t, wk, 1)
            for q in range(2, 4):
                for k in range(NKT):
                    dma_x(k, q)
            for p in (2, 3):
                dma_w(wq_t, wq, p)
                dma_w(wk_t, wk, p)
                dma_w(wv_t, wv, p)
            for p in range(NPAIR):
                nc.sync.dma_start(wo_p[p][0:64, :],
                                  wo[(2 * p) * 64:(2 * p) * 64 + 64, :])
                nc.sync.dma_start(wo_p[p][64:128, :],
                                  wo[(2 * p + 1) * 64:(2 * p + 1) * 64 + 64, :])
            for p in range(NPAIR):
                for j in range(NJT):
                    nc.vector.memset(VS[p][j][:], 1.0)

            with ExitStack() as es:
                st_ps = es.enter_context(
                    tc.tile_pool(name="stps", bufs=2, space="PSUM"))
                ct_ps = es.enter_context(
                    tc.tile_pool(name="ctps", bufs=2, space="PSUM"))
                mix_ps = es.enter_context(
                    tc.tile_pool(name="mixps", bufs=1, space="PSUM"))
                pt_pool = es.enter_context(
                    tc.tile_pool(name="ptpool", bufs=PVLAG + 1))
                nrm_pool = es.enter_context(tc.tile_pool(name="nrm", bufs=2))
                ctmp_pool = es.enter_context(tc.tile_pool(name="ctmp", bufs=2))
                posb_pool = es.enter_context(tc.tile_pool(name="posb", bufs=3))

                # ---------- A units (projections) ----------
                # One persistent 2-bank PSUM slot; A accumulators and C
                # output tiles ping-pong its 512-col halves (A drains by
                # ~stage 132, C starts ~stage 198 -- temporally disjoint,
                # range-based deps order the handoff).
                mslot = mix_ps.tile([P, 2 * MSEG], F32, tag="mix", name="mslot")
                a_ctr = [0]

                def emit_a1(p, c0, c1, ti):
                    wt = wq_t if ti == 0 else wk_t
                    dst = QT[p] if ti == 0 else KT[p]
                    ms = slice(c0, c1)
                    half = a_ctr[0] % 2
                    a_ctr[0] += 1
                    acc = mslot[:, half * MSEG:half * MSEG + (c1 - c0)]
                    for kt in range(NKT):
                        nc.tensor.matmul(
                            acc, lhsT=wt[:, kt, p * P:(p + 1) * P],
                            rhs=xsb[kt][:, ms],
                            start=(kt == 0), stop=(kt == NKT - 1))
                    nc.vector.tensor_copy(dst[:, ms], acc)

                def emit_a2(pp, jt):
                    """V projection for pair-pair pp (pairs 2pp, 2pp+1)."""
                    ts = slice(jt * P, (jt + 1) * P)
                    half = a_ctr[0] % 2
                    a_ctr[0] += 1
                    acc = mslot[:, half * MSEG:half * MSEG + 2 * P]
                    for kt in range(NKT):
                        nc.tensor.matmul(
                            acc, lhsT=xsb[kt][:, ts],
                            rhs=wv_t[:, kt, pp * 2 * P:(pp + 1) * 2 * P],
                            start=(kt == 0), stop=(kt == NKT - 1))
                    for i in range(2):
                        p = 2 * pp + i
                        vsv = VS[p][jt].rearrange("p (h c) -> p h c", c=VW)
                        acc_v = acc[:, i * P:(i + 1) * P].rearrange(
                            "p (h c) -> p h c", c=64)
                        nc.vector.tensor_copy(vsv[:, :, 0:64], acc_v)

                # ---------- B/C helpers ----------
                def emit_scores(p, qb, jt):
                    qs = slice(qb * QB, (qb + 1) * QB)
                    ks = slice(jt * P, (jt + 1) * P)
                    st = st_ps.tile([P, 2 * QB], F32, tag="st", name="st")
                    nc.tensor.matmul(st[:, 0:QB], lhsT=KT[p][0:64, ks],
                                     rhs=QT[p][0:64, qs], start=True, stop=True)
                    nc.tensor.matmul(st[:, QB:2 * QB], lhsT=KT[p][64:128, ks],
                                     rhs=QT[p][64:128, qs], start=True, stop=True)
                    return st

                def emit_pv(p, jt, ptp, cts):
                    nc.tensor.matmul(cts[0][0:65, :],
                                     lhsT=VS[p][jt][:, 0:VW],
                                     rhs=ptp[:, 0:QB],
                                     start=(jt == 0), stop=(jt == NJT - 1))
                    nc.tensor.matmul(cts[1][0:65, :],
                                     lhsT=VS[p][jt][:, VW:2 * VW],
                                     rhs=ptp[:, QB:2 * QB],
                                     start=(jt == 0), stop=(jt == NJT - 1))

                def emit_norm(p, qb, cts):
                    """Normalize both heads and pair-stack into cth[p][qb]:
                    even head -> partitions 0-63 (direct DVE), odd head ->
                    64-127 (via SBUF->SBUF DMA partition shift)."""
                    zst = nrm_pool.tile([1, 2 * QB], F32, tag="zst", name="zst")
                    nc.vector.tensor_copy(zst[0:1, 0:QB], cts[0][64:65, :])
                    nc.vector.tensor_copy(zst[0:1, QB:2 * QB], cts[1][64:65, :])
                    zr = nrm_pool.tile([1, 2 * QB], F32, tag="zr", name="zr")
                    nc.vector.reciprocal_approx_fast(zr[0:1, :], zst[0:1, :])
                    zrb = nrm_pool.tile([64, 2 * QB], F32, tag="zrb", name="zrb")
                    nc.gpsimd.partition_broadcast(zrb[:], zr[0:1, :])
                    nc.vector.tensor_tensor(cth[p][qb][0:64, :], cts[0][0:64, :],
                                            zrb[:, 0:QB], MULT)
                    ctmp = ctmp_pool.tile([64, QB], BF16, tag="ctmp", name="ctmp")
                    nc.vector.tensor_tensor(ctmp[:], cts[1][0:64, :],
                                            zrb[:, QB:2 * QB], MULT)
                    nc.sync.dma_start(cth[p][qb][64:128, :], ctmp[:])

                def emit_c_group(qb, gi, tail=False):
                    mtl, half = divmod(gi, 2)
                    mt = qb * 4 + mtl
                    ms = slice(mtl * P, (mtl + 1) * P)
                    hs = slice(half * 512, (half + 1) * 512)
                    pv = mslot[:, (gi % 2) * 512:(gi % 2) * 512 + 512]
                    for p in range(NPAIR):
                        nc.tensor.matmul(pv, lhsT=cth[p][qb][:, ms],
                                         rhs=wo_p[p][:, hs],
                                         start=(p == 0), stop=(p == NPAIR - 1))
                    po_sb = posb_pool.tile([P, 512], F32, tag="posb", name="po_sb")
                    if gi % 2 == 1:
                        nc.scalar.copy(po_sb[:], pv)
                    else:
                        nc.vector.tensor_copy(po_sb[:], pv)
                    nc.sync.dma_start(out[mt * P:(mt + 1) * P, hs], po_sb[:])

                # ---------- schedule ----------
                # A1(p0) msegs 0,1 fully up front (token half 0); the rest
                # drains into the stage loop, ordered so emission always
                # precedes the first consumer's emission (in-order PE queue).
                for ti in range(2):
                    emit_a1(0, 0, 256, ti)
                for ti in range(2):
                    emit_a1(0, 256, 512, ti)
                for ti in range(2):
                    emit_a1(0, 512, 1024, ti)

                # p0 drain order: each unit lands (at 1/stage) no later
                # than its first consumer's stage. KT mseg2 by stage 7
                # (scores jt8), mseg3 by 11 (scores jt12); a2(jt) by the
                # stage PV(jt) is popped.
                aq = []
                for jt in range(6):
                    aq.append(("a2", 0, jt))
                aq.append(("a1", 0, 1024, 1536, 0))
                aq.append(("a1", 0, 1024, 1536, 1))
                aq.append(("a2", 0, 6))
                aq.append(("a2", 0, 7))
                aq.append(("a1", 0, 1536, 2048, 0))
                aq.append(("a1", 0, 1536, 2048, 1))
                for jt in range(8, NJT):
                    aq.append(("a2", 0, jt))
                for mseg in range(NMSEG):
                    for ti in range(2):
                        aq.append(("a1", 1, mseg * MSEG, (mseg + 1) * MSEG, ti))
                for jt in range(NJT):
                    aq.append(("a2", 1, jt))
                for p in (2, 3):
                    for mseg in range(NMSEG):
                        for ti in range(2):
                            aq.append(("a1", p, mseg * MSEG, (mseg + 1) * MSEG, ti))

                # explicit drain schedule: p0's 20 units at 1/stage
                # (deadline-bound), A1(p1) every 4th stage (needed by 64),
                # then the rest every 3rd stage (a2(1) by 134, a1(2) by 128,
                # a1(3) by 192 -- all met with >=7 stages of margin).
                # p0: 1/stage (deadline-bound). A1(p1) 1-per-4 (done 48,
                # needed 64). a2(1) 1-per-3 (done 97, needed 134). a1(2)
                # 1-per-3 (done 121, needed 128). a1(3) 1-per-5 from 152
                # (done 187; scores(p3,qb0,jt) consume mseg jt//4 from
                # stage 192+jt, each evac lands >=5 stages ahead).
                drain_sidx = (list(range(20)) + list(range(20, 52, 4)) +
                              list(range(52, 100, 3)) + list(range(100, 124, 3)) +
                              list(range(152, 192, 5)))
                assert len(drain_sidx) >= len(aq)

                def drain_a(sidx_now):
                    while aq and drain_sidx[0] <= sidx_now:
                        drain_sidx.pop(0)
                        u = aq.pop(0)
                        if u[0] == "a1":
                            emit_a1(u[1], u[2], u[3], u[4])
                        else:
                            emit_a2(u[1], u[2])

                pv_queue = []  # (p, qb, jt, ptp, cts)

                def pop_pv():
                    pp, pqb, pj, pptp, pcts = pv_queue.pop(0)
                    emit_pv(pp, pj, pptp, pcts)
                    if pj == NJT - 1:
                        emit_norm(pp, pqb, pcts)

                sidx = 0
                for p in range(NPAIR):
                    for qb in range(NQB):
                        cts = None
                        for jt in range(NJT):
                            if jt == 0:
                                cts = [ct_ps.tile([P, QB], F32, tag="ct",
                                                  name="cte"),
                                       ct_ps.tile([P, QB], F32, tag="ct",
                                                  name="cto")]
                            st = emit_scores(p, qb, jt)
                            ptp = pt_pool.tile([P, 2 * QB], BF16,
                                               tag="pt", name="ptp")
                            if jt in dve_jts[p]:
                                nc.vector.tensor_scalar(
                                    ptp[:].bitcast(I16), st[:],
                                    SCHRA_A, SCHRA_B, MULT, ADD)
                            else:
                                nc.scalar.activation(ptp[:], st[:],
                                                     EXPF, scale=0.125)
                            pv_queue.append((p, qb, jt, ptp, cts))
                            # interleaved A units (emitted before PV pops so
                            # an A2 landing at its consumer's stage still
                            # precedes the consuming PV in program order)
                            drain_a(sidx)
                            # steady lag PVLAG; from the second window on,
                            # drain the whole window by its last stage so norm
                            # runs in-window and the ct banks are free before
                            # the next window's first PV.
                            if jt < 12 or (p, qb) == (0, 0):
                                if len(pv_queue) > PVLAG:
                                    pop_pv()
                            else:
                                npop = 2 if jt < 15 else 4
                                for _ in range(npop):
                                    if pv_queue:
                                        pop_pv()
                            # C groups during pair 3's windows. norm(qb-1)
                            # is emitted by the end-drain at jt==15 of the
                            # previous window, so groups spread from jt=3 on
                            # odd stages with no end-of-window burst.
                            if p == 3 and qb > 0:
                                if jt >= 3 and jt % 2 == 1:
                                    emit_c_group(qb - 1, (jt - 3) // 2)
                                elif jt == 14:
                                    emit_c_group(qb - 1, 6)
                            if p == 3 and qb > 0 and jt == 15:
                                emit_c_group(qb - 1, 7)
                            sidx += 1
                while pv_queue:
                    pop_pv()
                drain_sidx[:0] = [0] * len(aq)
                drain_a(10**9)

                # Tail: pairs-0..2 partial chains run during the final norm;
                # the norm-gated pair-3 MM + evac finish each group. Two
                # groups interleave across the mslot halves (same pattern
                # as the PV E/O chains).
                def c_partial(qb, gi):
                    mtl = gi // 2
                    ms = slice(mtl * P, (mtl + 1) * P)
                    hs = slice((gi % 2) * 512, (gi % 2) * 512 + 512)
                    pv = mslot[:, (gi % 2) * 512:(gi % 2) * 512 + 512]
                    for p in range(3):
                        nc.tensor.matmul(pv, lhsT=cth[p][qb][:, ms],
                                         rhs=wo_p[p][:, hs],
                                         start=(p == 0), stop=False)

                def c_final(qb, gi):
                    mtl, half = divmod(gi, 2)
                    mt = qb * 4 + mtl
                    ms = slice(mtl * P, (mtl + 1) * P)
                    hs = slice(half * 512, half * 512 + 512)
                    pv = mslot[:, half * 512:half * 512 + 512]
                    nc.tensor.matmul(pv, lhsT=cth[3][qb][:, ms],
                                     rhs=wo_p[3][:, hs],
                                     start=False, stop=True)
                    po_sb = posb_pool.tile([P, 512], F32, tag="posb",
                                           name="po_sb")
                    if gi % 2 == 1:
                        nc.scalar.copy(po_sb[:], pv)
                    else:
                        nc.vector.tensor_copy(po_sb[:], pv)
                    nc.sync.dma_start(out[mt * P:(mt + 1) * P, hs], po_sb[:])

                for gp in range(4):
                    c_partial(NQB - 1, 2 * gp)
                    c_partial(NQB - 1, 2 * gp + 1)
                    c_final(NQB - 1, 2 * gp)
                    c_final(NQB - 1, 2 * gp + 1)
    nc.compile()
    return nc


_NC_CACHE = {}


def _get_nc(S=2048, dve_mode=3):
    key = (S, dve_mode)
    if key not in _NC_CACHE:
        _NC_CACHE[key] = build_nc(S, dve_mode)
    return _NC_CACHE[key]


def make_in_maps(x, Wq, Wk, Wv, Wo):
    import ml_dtypes
    bf16 = ml_dtypes.bfloat16
    in_maps = []
    for c in range(8):
        b, g = divmod(c, 2)
        cols = slice(g * HD, (g + 1) * HD)
        in_maps.append({
            "xT": np.ascontiguousarray(x[b].T).astype(bf16),
            "wq": np.ascontiguousarray(Wq[:, cols]).astype(bf16),
            "wk": np.ascontiguousarray(Wk[:, cols]).astype(bf16),
            "wv": np.ascontiguousarray(Wv[:, cols]).astype(bf16),
            "wo": np.ascontiguousarray(Wo[cols, :]).astype(bf16),
        })
    return in_maps


def kernel(x, Wq, Wk, Wv, Wo, bo):
    from concourse.bass_utils import run_bass_kernel_spmd

    x = np.asarray(x, dtype=np.float32)
    Wq = np.asarray(Wq, dtype=np.float32)
    Wk = np.asarray(Wk, dtype=np.float32)
    Wv = np.asarray(Wv, dtype=np.float32)
    Wo = np.asarray(Wo, dtype=np.float32)
    bo = np.asarray(bo, dtype=np.float32)

    bs, S, d = x.shape
    nc = _get_nc(S)
    in_maps = make_in_maps(x, Wq, Wk, Wv, Wo)

    res = run_bass_kernel_spmd(nc, in_maps, core_ids=list(range(8)))
    outp = np.empty((bs, S, d), dtype=np.float32)
    for b in range(bs):
        outp[b] = res.results[2 * b]["out"] + res.results[2 * b + 1]["out"] + bo
    return outp
